# revision 2
# baseline (speedup 1.0000x reference)
"""3-branch GCN (DGL GraphConv x3 + max-pool + MLP head) on 8 TRN2 NeuronCores.

v2: dst-node sharding (2500/core). Per layer each core gathers all src rows of
a dst tile from a replicated DRAM table with ONE batched dma_gather (Ck*128
int16 idxs -> [128, Ck, D] SBUF tile), aggregates via one-hot fp16 matmuls into
PSUM with the full edge weight rsqrt(outdeg[src])*rsqrt(indeg[dst]) folded into
the one-hot values (host-precomputed per edge), applies the dense W matmul per
dst tile, and AllGathers layer output shards. Layer rounds are interleaved
across the 3 graphs so collectives hide under other graphs' compute. x is
converted f32->f16 sharded (2500 rows/core) and AllGathered once per graph.
Max-pool is local + AllReduce(max); the tiny MLP head runs replicated.
"""
import numpy as np
import ml_dtypes
import concourse.bass as bass
import concourse.bacc as bacc
import concourse.tile as tile
import concourse.mybir as mybir
from concourse.bass_utils import run_bass_kernel_spmd
from concourse.library_config import mlp as mlp_lib

NC_ = 8
N = 20000
E = 320000
SH = N // NC_          # 2500 nodes per core
NT = 20                # dst tiles per core (19 full + 68-node partial)
D_IN, D_H = 128, 304
DPAD = 384             # f16 row pad -> 768B rows (dma_gather needs 256B mult)
f16, f32 = mybir.dt.float16, mybir.dt.float32
f8 = mybir.dt.float8e4
i16, i32 = mybir.dt.int16, mybir.dt.int32
AF = mybir.ActivationFunctionType
ALU = mybir.AluOpType
core_ids = list(range(NC_))

# S-build engine split: chunks with (idx % MOD) < CNT go to gpsimd
GP_SBUILD_MOD = 3
GP_SBUILD_CNT = 1


def _prep_graph(src, dst):
    """Per-core chunked edge metadata with core-uniform chunk counts."""
    src = np.asarray(src).astype(np.int64)
    dst = np.asarray(dst).astype(np.int64)
    outdeg = np.bincount(src, minlength=N).clip(1).astype(np.float32)
    indeg = np.bincount(dst, minlength=N).clip(1).astype(np.float32)
    ro_full = (1.0 / np.sqrt(outdeg)).astype(np.float32)
    ri_full = (1.0 / np.sqrt(indeg)).astype(np.float32)
    per_core = []
    for c in range(NC_):
        m = (dst // SH) == c
        es, ed = src[m], dst[m] - c * SH
        tiles = []
        for t in range(NT):
            tm = (ed // 128) == t
            tiles.append((es[tm], ed[tm] - t * 128))
        per_core.append(tiles)
    Ck = [max(int(np.ceil(len(per_core[c][t][0]) / 128)) for c in range(NC_)) or 1
          for t in range(NT)]
    nchunks = sum(Ck)
    idxs = np.zeros((NC_, 128, nchunks * 8), np.int16)     # wrap-16; pad row 0
    smat = np.zeros((NC_, 128, nchunks, 128), ml_dtypes.float8_e4m3)
    ri = np.ones((NC_, 128, NT), np.float32)               # rsqrt(indeg), shard
    ro = np.ones((NC_, 128, NT), np.float32)               # rsqrt(outdeg), shard
    for c in range(NC_):
        j0 = 0
        for t in range(NT):
            es, er = per_core[c][t]
            npad = Ck[t] * 128
            e_s = np.zeros(npad, np.int64)
            e_s[:len(es)] = es
            # idx j gathers to out[j % 128, j // 128, :]; idx j lives at
            # sbuf[j % 16, j // 16] replicated across the 8 groups of 16
            wrapped = e_s.reshape(-1, 16).T.astype(np.int16)   # [16, Ck*8]
            idxs[c, :, j0 * 8:(j0 + Ck[t]) * 8] = np.tile(wrapped, (8, 1))
            # host-built one-hot S: chunk c_, edge slot p -> dst column
            sm = np.zeros((npad, 128), np.float32)
            sm[np.arange(len(er)), er] = 1.0
            smat[c, :, j0:j0 + Ck[t], :] = (
                sm.reshape(Ck[t], 128, 128).transpose(1, 0, 2)
                .astype(ml_dtypes.float8_e4m3))
            j0 += Ck[t]
            lo = c * SH + t * 128
            hi = min(lo + 128, (c + 1) * SH)
            ri[c, :hi - lo, t] = ri_full[lo:hi]
            ro[c, :hi - lo, t] = ro_full[lo:hi]
    return Ck, idxs, smat, ri, ro


def _build(g_meta):
    nc = bacc.Bacc(None, target_bir_lowering=False)
    ext = {}
    for g in range(3):
        Ck, idxs, smat, ri, ro = g_meta[g]
        nch = sum(Ck)
        ext[f"xs32{g}"] = nc.dram_tensor(f"xs32{g}", [SH, D_IN], f32, kind="ExternalInput")
        ext[f"ix{g}"] = nc.dram_tensor(f"ix{g}", [128, nch * 8], i16, kind="ExternalInput")
        ext[f"sm{g}"] = nc.dram_tensor(f"sm{g}", [128, nch * 128], f8, kind="ExternalInput")
        ext[f"ri{g}"] = nc.dram_tensor(f"ri{g}", [128, NT], f32, kind="ExternalInput")
        ext[f"ro{g}"] = nc.dram_tensor(f"ro{g}", [128, NT], f32, kind="ExternalInput")
    for nm, shp in [("W1", [D_IN, D_H]), ("W2", [D_H, D_H]), ("W3", [D_H, D_H]),
                    ("b1", [1, D_H]), ("b2", [1, D_H]), ("b3", [1, D_H]),
                    ("fW1", [D_H, 128]), ("fb1", [1, 128]), ("fW2", [128, 64]),
                    ("fb2", [1, 64]), ("fW3", [64, 1]), ("fb3", [1, 1])]:
        ext[nm] = nc.dram_tensor(nm, shp, f32, kind="ExternalInput")
    y_ext = nc.dram_tensor("y", [1, 1], f32, kind="ExternalOutput")

    ident_d = nc.inline_tensor(np.eye(128, dtype=np.float32), name="ident")
    ones16_d = nc.inline_tensor(np.ones((1, 128), np.float16), name="ones16")
    ones32_d = nc.inline_tensor(np.ones((1, 1), np.float32), name="ones32")

    with tile.TileContext(nc) as tc:
        with (
            tc.tile_pool(name="cst", bufs=1) as cst,
            tc.tile_pool(name="meta", bufs=1) as meta,
            tc.tile_pool(name="g", bufs=3) as gp,
            tc.tile_pool(name="s", bufs=6) as sp,
            tc.tile_pool(name="z", bufs=3) as zp,
            tc.tile_pool(name="a", bufs=3) as apool,
            tc.tile_pool(name="h", bufs=3) as hp,
            tc.tile_pool(name="x", bufs=3) as xp_pool,
            tc.tile_pool(name="ps", bufs=2, space="PSUM") as pp,
            tc.tile_pool(name="ps2", bufs=2, space="PSUM") as pp2,
            tc.tile_pool(name="pst", bufs=2, space="PSUM") as tpp,
            tc.tile_pool(name="dram", bufs=1, space="DRAM") as dram,
        ):
            nc.gpsimd.load_library(mlp_lib)
            ident_t = cst.tile([128, 128], f32)
            nc.sync.dma_start(ident_t[:], ident_d[:])
            ones16 = cst.tile([1, 128], f16)
            nc.sync.dma_start(ones16[:], ones16_d[:])
            ones32 = cst.tile([1, 1], f32)
            nc.sync.dma_start(ones32[:], ones32_d[:])

            # weights resident (f16 for the graph convs, f32 for the head)
            W_t = {}
            w1t = cst.tile([128, D_H], f16, name="w1t")
            W_t[1] = [w1t]
            nc.gpsimd.dma_start(W_t[1][0][:], ext["W1"][:])
            for L in (2, 3):
                W_t[L] = []
                for j in range(3):
                    k = 128 if j < 2 else 48
                    w = cst.tile([128, D_H], f16, name=f"w{L}_{j}")
                    nc.gpsimd.dma_start(w[0:k, :], ext[f"W{L}"][j * 128:j * 128 + k, :])
                    W_t[L].append(w)
            b_t = {}
            for L in (1, 2, 3):
                b = cst.tile([1, D_H], f16, name=f"b{L}t")
                nc.gpsimd.dma_start(b[:], ext[f"b{L}"][:])
                b_t[L] = b
            fW1_t = []
            for j in range(3):
                k = 128 if j < 2 else 48
                w = cst.tile([128, 128], f32, name=f"fw1_{j}")
                nc.sync.dma_start(w[0:k, :], ext["fW1"][j * 128:j * 128 + k, :])
                fW1_t.append(w)
            fW2_t = cst.tile([128, 64], f32)
            nc.sync.dma_start(fW2_t[:], ext["fW2"][:])
            fW3_t = cst.tile([64, 1], f32)
            nc.sync.dma_start(fW3_t[:], ext["fW3"][:])
            fb_t = {}
            for nm, w in [("fb1", 128), ("fb2", 64), ("fb3", 1)]:
                b = cst.tile([1, w], f32, name=f"{nm}t")
                nc.sync.dma_start(b[:], ext[nm][:])
                fb_t[nm] = b

            # per-graph edge metadata (resident; graphs are interleaved)
            ix_t, ri_t, ro_t = {}, {}, {}
            for g in range(3):
                nch = sum(g_meta[g][0])
                ix_t[g] = meta.tile([128, nch * 8], i16, name=f"ix{g}")
                nc.sync.dma_start(ix_t[g][:], ext[f"ix{g}"][:])
                ri_t[g] = meta.tile([128, NT], f32, name=f"ri{g}")
                nc.sync.dma_start(ri_t[g][:], ext[f"ri{g}"][:])
                ro_t[g] = meta.tile([128, NT], f32, name=f"ro{g}")
                nc.sync.dma_start(ro_t[g][:], ext[f"ro{g}"][:])

            # DRAM tables: xp (f16 x), hfA/hfB (padded layer outputs)
            xp, hfA, hfB, xsh, shard_b = [], [], [], [], []
            for g in range(3):
                xp.append(dram.tile([N, D_IN], f16, addr_space="Shared", name=f"xp{g}"))
                hfA.append(dram.tile([N, DPAD], f16, addr_space="Shared", name=f"hfA{g}"))
                hfB.append(dram.tile([N, DPAD], f16, addr_space="Shared", name=f"hfB{g}"))
                xsh.append(dram.tile([SH, D_IN], f16, name=f"xsh{g}"))
                shard_b.append(dram.tile([SH, DPAD], f16, name=f"shb{g}"))
            pool_in = dram.tile([128, 3], f32)
            pool_out = dram.tile([128, 3], f32, addr_space="Shared")
            vec_b = dram.tile([1, 128], f32)

            macc = cst.tile([128, D_H], f32)
            nc.vector.memset(macc[:], 0.0)

            ck_max = max(max(g_meta[g][0]) for g in range(3))

            # stage 0: sharded x f32->f16 convert + AllGather, interleaved
            # (each core receives only its own shard rows as input xs32{g})
            for g in range(3):
                for t in range(NT):
                    rows = 128 if t < NT - 1 else SH - (NT - 1) * 128
                    xt = xp_pool.tile([128, D_IN], f32, tag="xt")
                    nc.sync.dma_start(
                        xt[0:rows, :],
                        ext[f"xs32{g}"][t * 128:t * 128 + rows, :])
                    xs = xp_pool.tile([128, D_IN], f16, tag="xs")
                    nc.scalar.activation(xs[0:rows, :], xt[0:rows, :], AF.Copy,
                                         scale=ro_t[g][0:rows, t:t + 1])
                    nc.sync.dma_start(xsh[g][t * 128:t * 128 + rows, :], xs[0:rows, :])
                nc.gpsimd.collective_compute(
                    "AllGather", ALU.bypass, replica_groups=[core_ids],
                    ins=[xsh[g].opt()], outs=[xp[g].opt()])

            sb_ctr = [0]

            def do_layer(g, L):
                Ck = g_meta[g][0]
                src_tab = xp[g] if L == 1 else (hfA[g] if L == 2 else hfB[g])
                DL = D_IN if L == 1 else D_H
                DLP = D_IN if L == 1 else DPAD
                j0 = 0
                for t in range(NT):
                    rows = 128 if t < NT - 1 else SH - (NT - 1) * 128
                    Ckt = Ck[t]
                    gt = gp.tile([128, ck_max, DLP], f16, tag=f"g{L}")
                    for c0 in range(0, Ckt, 8):
                        cnt = min(8, Ckt - c0)
                        nc.gpsimd.dma_gather(
                            gt[:, c0:c0 + cnt, :], src_tab[:],
                            ix_t[g][:, (j0 + c0) * 8:(j0 + c0 + cnt) * 8],
                            cnt * 128, cnt * 128, DLP, single_packet=True)
                    s_sb = sp.tile([128, Ckt, 128], f8, tag="s")
                    nc.sync.dma_start(
                        s_sb[:],
                        ext[f"sm{g}"][:, j0 * 128:(j0 + Ckt) * 128]
                        .rearrange("p (c e) -> p c e", e=128))
                    psum = pp.tile([128, D_H], f32, tag="agg")
                    for c in range(Ckt):
                        nc.tensor.matmul(psum[:, 0:DL], s_sb[:, c, :],
                                         gt[:, c, 0:DL],
                                         start=(c == 0), stop=(c == Ckt - 1))
                    j0 += Ckt
                    # dense W matmul per dst tile (transpose agg, then W)
                    zsb = zp.tile([128, D_H], f32, tag="zsb")
                    nc.scalar.activation(zsb[:, 0:DL], psum[:, 0:DL], AF.Copy,
                                         scale=ri_t[g][:, t:t + 1])
                    psum2 = pp2.tile([128, D_H], f32, tag="wout")
                    J = 1 if L == 1 else 3
                    for j in range(J):
                        k = 128 if (j < J - 1 or L == 1) else 48
                        tp = tpp.tile([128, 128], f32, tag="tp")
                        nc.tensor.transpose(tp[0:k, :], zsb[:, j * 128:j * 128 + k],
                                            ident_t[:])
                        at = apool.tile([128, 128], f16, tag="at")
                        nc.scalar.activation(at[0:k, :], tp[0:k, :], AF.Copy)
                        nc.tensor.matmul(psum2[:], at[0:k, :], W_t[L][j][0:k, :],
                                         start=(j == 0), stop=False)
                    nc.tensor.matmul(psum2[:], ones16[:], b_t[L][:],
                                     start=False, stop=True)
                    if L < 3:
                        hsb = hp.tile([128, D_H], f16, tag="hsb")
                        nc.scalar.activation(hsb[:], psum2[:], AF.Relu,
                                             scale=ro_t[g][:, t:t + 1])
                        nc.sync.dma_start(
                            shard_b[g][t * 128:t * 128 + rows, 0:D_H], hsb[0:rows, :])
                    else:
                        hsb = hp.tile([128, D_H], f32, tag="hsb3")
                        nc.scalar.activation(hsb[:], psum2[:], AF.Relu)
                        nc.vector.tensor_tensor(macc[0:rows, :], macc[0:rows, :],
                                                hsb[0:rows, :], ALU.max)

            for L in (1, 2, 3):
                for g in range(3):
                    do_layer(g, L)
                    if L < 3:
                        dstf = hfA[g] if L == 1 else hfB[g]
                        nc.gpsimd.collective_compute(
                            "AllGather", ALU.bypass,
                            replica_groups=[core_ids],
                            ins=[shard_b[g].opt()],
                            outs=[dstf.opt()])

            # max over partitions via transpose + reduce, AllReduce, MLP
            pool_sb = cst.tile([128, 3], f32)
            for j in range(3):
                k = 128 if j < 2 else 48
                tp = tpp.tile([128, 128], f32, tag="tp")
                nc.tensor.transpose(tp[0:k, :], macc[:, j * 128:j * 128 + k], ident_t[:])
                nc.vector.tensor_reduce(pool_sb[0:k, j:j + 1], tp[0:k, :],
                                        mybir.AxisListType.X, ALU.max)
            nc.sync.dma_start(pool_in[:], pool_sb[:])
            nc.gpsimd.collective_compute(
                "AllReduce", ALU.max, replica_groups=[core_ids],
                ins=[pool_in.opt()], outs=[pool_out.opt()])
            pool_t = cst.tile([128, 3], f32)
            nc.sync.dma_start(pool_t[:], pool_out[:])

            z1p = pp2.tile([1, 128], f32, tag="z")
            for j in range(3):
                k = 128 if j < 2 else 48
                nc.tensor.matmul(z1p[:], pool_t[0:k, j:j + 1], fW1_t[j][0:k, :],
                                 start=(j == 0), stop=False)
            nc.tensor.matmul(z1p[:], ones32[:], fb_t["fb1"][:], start=False, stop=True)
            z1s = cst.tile([1, 128], f32)
            nc.scalar.activation(z1s[:], z1p[:], AF.Relu)
            nc.sync.dma_start(vec_b[:], z1s[:])
            z1T = cst.tile([128, 1], f32)
            nc.sync.dma_start(z1T[:], vec_b[0, :].rearrange("(p o) -> p o", o=1))
            z2p = pp2.tile([1, 64], f32, tag="z")
            nc.tensor.matmul(z2p[:], z1T[:], fW2_t[:], start=True, stop=False)
            nc.tensor.matmul(z2p[:], ones32[:], fb_t["fb2"][:], start=False, stop=True)
            z2s = cst.tile([1, 64], f32)
            nc.scalar.activation(z2s[:], z2p[:], AF.Relu)
            nc.sync.dma_start(vec_b[0:1, 0:64], z2s[:])
            z2T = cst.tile([64, 1], f32)
            nc.sync.dma_start(z2T[:], vec_b[0, 0:64].rearrange("(p o) -> p o", o=1))
            z3p = pp2.tile([1, 1], f32, tag="z")
            nc.tensor.matmul(z3p[:], z2T[:], fW3_t[:], start=True, stop=False)
            nc.tensor.matmul(z3p[:], ones32[:], fb_t["fb3"][:], start=False, stop=True)
            ys = cst.tile([1, 1], f32)
            nc.scalar.activation(ys[:], z3p[:], AF.Sigmoid)
            nc.sync.dma_start(y_ext[:], ys[:])

    nc.compile()
    return nc


RUN_KWARGS = {}
LAST_RES = None


def kernel(**inputs):
    g_meta = []
    for g, (s, d) in enumerate([("src1", "dst1"), ("src2", "dst2"), ("src3", "dst3")]):
        g_meta.append(_prep_graph(inputs[s], inputs[d]))
    nc = _build(g_meta)
    in_maps = []
    for c in range(NC_):
        m = {}
        for g, xn in enumerate(["x1", "x2", "x3"]):
            Ck, idxs, smat, ri, ro = g_meta[g]
            nch = sum(Ck)
            xfull = np.asarray(inputs[xn], np.float32)
            m[f"xs32{g}"] = np.ascontiguousarray(xfull[c * SH:(c + 1) * SH])
            m[f"ix{g}"] = idxs[c]
            m[f"sm{g}"] = np.ascontiguousarray(smat[c].reshape(128, nch * 128))
            m[f"ri{g}"] = ri[c]
            m[f"ro{g}"] = ro[c]
        for nm in ["W1", "W2", "W3", "fW2"]:
            m[nm] = np.asarray(inputs[nm], np.float32)
        m["fW1"] = np.asarray(inputs["fW1"], np.float32)
        m["fW3"] = np.asarray(inputs["fW3"], np.float32).reshape(64, 1)
        for nm in ["b1", "b2", "b3", "fb1", "fb2", "fb3"]:
            m[nm] = np.asarray(inputs[nm], np.float32).reshape(1, -1)
        in_maps.append(m)
    res = run_bass_kernel_spmd(nc, in_maps, core_ids, **RUN_KWARGS)
    global LAST_RES
    LAST_RES = res
    return np.asarray(res.results[0]["y"], np.float32).reshape(1)
